# revision 2
# baseline (speedup 1.0000x reference)
"""2D Haar DWT (pywt 'haar' dwt2) on 8 Trainium2 NeuronCores via Bass/Tile.

Input:  x [16, 64, 256, 256] f32
Output: (LL, LH, HL, HH), each [16, 64, 128, 128] f32, matching
        LL = (a+b+c+d)/2 etc. per 2x2 block [[a, b], [c, d]].

Sharding: batch dim 16 -> 2 per core across 8 cores, no communication.

Per-core plan: 128 images, processed in groups of 8. One contiguous 2 MB
DMA brings [128 pair-rows, 8 imgs, 512] into SBUF (free dim = img x
(top_row | bot_row)). DVE computes row sum/diff, ACT scales by 0.5, DVE
does the stride-2 column butterfly, four 512 KB DMAs store the quadrants.
"""

from contextlib import ExitStack

import numpy as np

SHARD_B, C, H, W = 2, 64, 256, 256
IMGS = SHARD_B * C          # 128 images per core
HP, WH = H // 2, W // 2
GROUP_IMGS = 8
N_CORES = 8
OUT_NAMES = ("ll", "lh", "hl", "hh")


def _build_nc(bufs: int = 3, group_imgs: int = GROUP_IMGS):
    import concourse.bacc as bacc
    import concourse.mybir as mybir
    import concourse.tile as tile

    n_groups = IMGS // group_imgs
    nc = bacc.Bacc()
    x = nc.dram_tensor("x", [SHARD_B, C, H, W], mybir.dt.float32, kind="ExternalInput")
    outs = {
        n: nc.dram_tensor(n, [SHARD_B, C, HP, WH], mybir.dt.float32, kind="ExternalOutput")
        for n in OUT_NAMES
    }
    xg = x[:, :, :, :].rearrange("b c (hp two) w -> (b c) hp (two w)", two=2)
    og = {k: v[:, :, :, :].rearrange("b c k w -> (b c) k w") for k, v in outs.items()}

    with tile.TileContext(nc) as tc, ExitStack() as ctx:
        xpool = ctx.enter_context(tc.tile_pool(name="xin", bufs=bufs))
        spool = ctx.enter_context(tc.tile_pool(name="srow", bufs=bufs))
        dpool = ctx.enter_context(tc.tile_pool(name="drow", bufs=bufs))
        opool = ctx.enter_context(tc.tile_pool(name="outs", bufs=bufs))
        for g in range(n_groups):
            j0, j1 = g * group_imgs, (g + 1) * group_imgs
            xt = xpool.tile([HP, group_imgs, 2 * W], mybir.dt.float32, tag="xt")
            nc.sync.dma_start(
                out=xt[:, :, :], in_=xg[j0:j1].rearrange("j p tw -> p j tw")
            )
            top = xt[:, :, 0:W]
            bot = xt[:, :, W : 2 * W]
            st = spool.tile([HP, group_imgs, W], mybir.dt.float32, tag="st")
            dt = dpool.tile([HP, group_imgs, W], mybir.dt.float32, tag="dt")
            nc.vector.tensor_add(st[:, :, :], top, bot)
            nc.vector.tensor_sub(dt[:, :, :], top, bot)
            nc.scalar.mul(st[:, :, :], st[:, :, :], 0.5)
            nc.scalar.mul(dt[:, :, :], dt[:, :, :], 0.5)
            views = {
                "ll": (st, "add"),
                "hl": (st, "sub"),
                "lh": (dt, "add"),
                "hh": (dt, "sub"),
            }
            for name, (srct, op) in views.items():
                ev = srct[:, :, 0:W:2]
                od = srct[:, :, 1:W:2]
                ot = opool.tile([HP, group_imgs, WH], mybir.dt.float32, tag=name)
                if op == "add":
                    nc.vector.tensor_add(ot[:, :, :], ev, od)
                else:
                    nc.vector.tensor_sub(ot[:, :, :], ev, od)
                nc.sync.dma_start(
                    out=og[name][j0:j1].rearrange("j k w -> k j w"), in_=ot[:, :, :]
                )
    nc.compile()
    return nc


_NC_CACHE = None


def _get_nc():
    global _NC_CACHE
    if _NC_CACHE is None:
        _NC_CACHE = _build_nc()
    return _NC_CACHE


def run_sharded(x: np.ndarray, trace: bool = False):
    """Run the SPMD kernel; returns (BassKernelResults, outputs dict of full arrays)."""
    from concourse.bass_utils import run_bass_kernel_spmd

    x = np.ascontiguousarray(x, dtype=np.float32)
    nc = _get_nc()
    in_maps = [
        {"x": x[i * SHARD_B : (i + 1) * SHARD_B]} for i in range(N_CORES)
    ]
    br = run_bass_kernel_spmd(nc, in_maps, list(range(N_CORES)), trace=trace)
    full = {}
    for name in OUT_NAMES:
        full[name] = np.concatenate(
            [np.asarray(br.results[i][name]).reshape(SHARD_B, C, HP, WH)
             for i in range(N_CORES)],
            axis=0,
        )
    return br, full


def kernel(x: np.ndarray):
    _, full = run_sharded(x, trace=False)
    return full["ll"], full["lh"], full["hl"], full["hh"]
